# revision 29
# baseline (speedup 1.0000x reference)
"""Trainium2 Bass kernel for nn_CheckinEncoder (2-layer GCN, PReLU between).

Math (per GCNConv layer, PyG semantics):
    deg[d]  = sum_{e: dst_e=d} w_e + 1                (weighted in-degree + self loop)
    dis     = deg^{-1/2}
    norm_e  = dis[src_e] * w_e * dis[dst_e]           (self loop: 1/deg[d])
    agg     = scatter_add(norm_e * x[src_e] -> dst_e) (aggregate-first; linear
    h       = agg @ W.T + b                            and aggregation commute)

Sharding: dst nodes row-partitioned across 8 cores (6250 each). Each core
owns the edges into its nodes. Layer 1 gathers from a replicated fp16 copy
of x; between layers an on-device AllGather replicates h; layer 2 gathers
from that. Aggregation is done on the TensorEngine: for each 128-edge chunk
a selection matrix S[e, j] = (j == dst_local_e) * norm_e is built on the
VectorEngine (iota compare) and aggT[f, dst] += G[:, f].T @ S accumulates in
PSUM directly in the transposed layout the dense matmul needs as lhsT.

dma_gather indices are int16, so source windows are split at row 32768
("lo": src < 32768, base 0; "hi": src >= 32768, base 32768). Per (tile,
half) the chunk count is fixed globally (max over cores/tiles) and padded
with (idx=0, norm=0) so the single SPMD program fits every core.
"""

import numpy as np

# ---------------------------------------------------------------- problem dims
N_NODES = 50000
IN_CH = 256
HID = 512
N_CORES = 8
P = 128
LO_SPLIT = 32768
FP16 = True  # gather/matmul datapath dtype (fp32 accumulation throughout)
NQ = 4       # SWDGE queues used round-robin for gathers


# ------------------------------------------------------------------ tile patch
# This container's walrus accepts at most 1 sync wait per instruction
# (2 for EventSemaphore); Tile can emit more. Two fixes: split the kernel-tail
# drain's waits across nops, and legalize the final BIR by hoisting excess
# waits onto inserted NoOps (same engine, just before the instruction).
_PATCHED = False


def _apply_patches():
    global _PATCHED
    if _PATCHED:
        return
    _PATCHED = True
    import concourse.mybir as mybir
    import concourse.tile as tile
    import concourse.bass2jax as bass2jax
    import concourse.bass_utils as bass_utils
    from concourse.vector_clock import ScopedClock

    def _patched_drain_and_barrier(self, tick_clock, wait_clock):
        nc = self.nc
        drain_inst = nc.sync.drain()
        wait_clock.add_sem_waits(
            drain_inst.ins, ScopedClock({None: tick_clock.global_clock})
        )
        waits = list(drain_inst.ins.sync_info.on_wait)
        if len(waits) > 1:
            drain_inst.ins.sync_info.on_wait = waits[:1]
            for w in waits[1:]:
                nop = nc.sync.nop(nofuse=True, hint="drain_split_wait")
                if nop.ins.sync_info is None:
                    nop.ins.sync_info = mybir.SyncInfo(on_wait=[w], on_update=[])
                else:
                    nop.ins.sync_info.on_wait = [w]
        nc.all_engine_barrier()
        assert self.sems is not None
        popped = nc._tile_sem_poison_stack.pop()
        assert popped is self._sem_poison
        nc.clear_and_free_semaphores(list(self.sems.allocated().values()))
        nc.all_engine_barrier()

    tile.TileContext._drain_and_barrier = _patched_drain_and_barrier

    def _legalize_bir_json(bir_bytes):
        import orjson

        m = orjson.loads(bir_bytes)
        for fn in m.get("functions", []):
            for blk in fn.get("blocks", []):
                out = []
                for inst in blk.get("instructions", []):
                    si = inst.get("sync_info")
                    cap = 2 if inst.get("opcode") == "EventSemaphore" else 1
                    if si and len(si.get("on_wait") or []) > cap:
                        waits = si["on_wait"]
                        for k, w in enumerate(waits[:-cap]):
                            out.append(
                                {
                                    "debug": inst.get("debug", 0),
                                    "engine": inst["engine"],
                                    "ins": [],
                                    "outs": [],
                                    "name": f"{inst['name']}-lw{k}",
                                    "opcode": "NoOp",
                                    "sync_info": {"on_update": [], "on_wait": [w]},
                                }
                            )
                        si["on_wait"] = waits[-cap:]
                    out.append(inst)
                blk["instructions"] = out
        return orjson.dumps(m)

    orig = bass_utils.compile_bir_kernel

    def _wrapped(bir_json, tmpdir, neff_name="file.neff", **kw):
        return orig(_legalize_bir_json(bir_json), tmpdir, neff_name, **kw)

    bass_utils.compile_bir_kernel = _wrapped
    bass2jax.compile_bir_kernel = _wrapped


# ------------------------------------------------------------ host preprocessing
def _edge_buckets(edge_index, edge_weight, n_nodes, n_cores):
    """Bucket (src, dst_local, norm) per (core, tile), self loops included."""
    npc = n_nodes // n_cores
    tiles = (npc + P - 1) // P

    src = np.asarray(edge_index[0], dtype=np.int64)
    dst = np.asarray(edge_index[1], dtype=np.int64)
    w = np.asarray(edge_weight, dtype=np.float32)

    deg = np.bincount(dst, weights=w.astype(np.float64), minlength=n_nodes)
    deg = deg.astype(np.float32) + 1.0  # + self loop weight
    dis = 1.0 / np.sqrt(deg)
    norm = (dis[src] * w * dis[dst]).astype(np.float32)

    loop = np.arange(n_nodes, dtype=np.int64)
    src = np.concatenate([src, loop])
    dst = np.concatenate([dst, loop])
    norm = np.concatenate([norm, (dis * dis).astype(np.float32)])

    core_of = dst // npc
    tile_of = (dst % npc) // P
    dloc = (dst % npc) % P

    per_ct = {}
    for c in range(n_cores):
        cm = core_of == c
        for t in range(tiles):
            m = cm & (tile_of == t)
            per_ct[(c, t)] = (src[m], dloc[m], norm[m])
    return per_ct, tiles, npc


def _pack(per_ct, n_cores, tiles, srcmap, lo_split):
    """Pack bucketed edges into fixed-shape gather idx + (dst,norm) meta
    arrays. `srcmap` maps original src node id -> gather row id."""
    max_lo, max_hi = 1, 0
    split = {}
    for key, (s, d, n) in per_ct.items():
        g = srcmap[s] if srcmap is not None else s
        order = np.argsort(g, kind="stable")
        g, d2, n2 = g[order], d[order], n[order]
        lo = g < lo_split
        split[key] = (g[lo], d2[lo], n2[lo], g[~lo] - lo_split, d2[~lo], n2[~lo])
        max_lo = max(max_lo, int(lo.sum()))
        max_hi = max(max_hi, int((~lo).sum()))

    CL = (max_lo + P - 1) // P
    CH = (max_hi + P - 1) // P
    cpt = CL + CH

    idx_arrays, meta_arrays = [], []
    for c in range(n_cores):
        idx_flat = np.zeros((tiles * cpt * P,), np.int16)
        meta = np.zeros((P, tiles * cpt * 2), np.float32)
        for t in range(tiles):
            slo, dlo, nlo, shi, dhi, nhi = split[(c, t)]
            base = t * cpt * P
            idx_flat[base : base + len(slo)] = slo.astype(np.int16)
            idx_flat[base + CL * P : base + CL * P + len(shi)] = shi.astype(np.int16)
            dcol = np.zeros((cpt * P,), np.float32)
            ncol = np.zeros((cpt * P,), np.float32)
            dcol[: len(dlo)] = dlo
            ncol[: len(nlo)] = nlo
            dcol[CL * P : CL * P + len(dhi)] = dhi
            ncol[CL * P : CL * P + len(nhi)] = nhi
            for ch in range(cpt):
                m = t * cpt + ch
                meta[:, 2 * m] = dcol[ch * P : (ch + 1) * P]
                meta[:, 2 * m + 1] = ncol[ch * P : (ch + 1) * P]
        idxw = idx_flat.reshape(-1, 16).T
        idx_arrays.append(np.tile(idxw, (8, 1)).copy())
        meta_arrays.append(meta)
    return idx_arrays, meta_arrays, CL, CH


# AllGather chunking: TG tiles of h per collective chunk, overlapped with
# remaining layer-1 compute. hfull layout is chunk-major: chunk c holds
# [rank 0 rows | rank 1 rows | ...]; gpos() maps node id -> hfull row.
TG = 7


def _chunk_sizes(tiles, npc):
    sizes = []
    for c0 in range(0, tiles, TG):
        lo = c0 * P
        hi = min((c0 + TG) * P, npc)
        sizes.append(hi - lo)
    return sizes


def _gpos_map(n_nodes, n_cores, tiles, npc):
    sizes = _chunk_sizes(tiles, npc)
    out_base = np.cumsum([0] + [8 * s for s in sizes])[:-1]
    n = np.arange(n_nodes, dtype=np.int64)
    r = n // npc
    l = n % npc
    c = np.minimum(l // (TG * P), len(sizes) - 1)
    s_c = np.asarray(sizes, np.int64)[c]
    return out_base[c] + r * s_c + (l - c * TG * P)


# ------------------------------------------------------------------ bass kernel
def _build(n_nodes, in_ch, hid, n_cores, lo_split, CLH1, CLH2, tiles, npc, prelu_a,
           reps=1, profile_mode=False, no_gather=False):
    import concourse.bacc as bacc
    import concourse.mybir as mybir
    import concourse.tile as tile

    from concourse.masks import make_identity

    dt = mybir.dt
    DT = dt.float16 if FP16 else dt.float32
    CL1, CH1 = CLH1
    CL2, CH2 = CLH2
    cpt1 = CL1 + CH1
    cpt2 = CL2 + CH2
    fb1 = in_ch // P   # feature blocks, layer-1 aggregation
    fb2 = hid // P
    last_rows = npc - (tiles - 1) * P
    csizes = _chunk_sizes(tiles, npc)
    out_base = np.cumsum([0] + [n_cores * s for s in csizes])[:-1]

    nc = bacc.Bacc(
        "TRN2", target_bir_lowering=False, num_devices=n_cores,
        num_swdge_queues=NQ,
    )
    x16 = nc.dram_tensor("x16", [n_nodes, in_ch], DT, kind="ExternalInput")
    idx1 = nc.dram_tensor("idx1", [P, tiles * cpt1 * P // 16], dt.int16, kind="ExternalInput")
    meta1 = nc.dram_tensor("meta1", [P, tiles * cpt1 * 2], dt.float32, kind="ExternalInput")
    idx2 = nc.dram_tensor("idx2", [P, tiles * cpt2 * P // 16], dt.int16, kind="ExternalInput")
    meta2 = nc.dram_tensor("meta2", [P, tiles * cpt2 * 2], dt.float32, kind="ExternalInput")
    w1t = nc.dram_tensor("w1t", [P, fb1 * hid], DT, kind="ExternalInput")
    w2t = nc.dram_tensor("w2t", [P, fb2 * hid], DT, kind="ExternalInput")
    b1f = nc.dram_tensor("b1f", [P, hid], dt.float32, kind="ExternalInput")
    b2f = nc.dram_tensor("b2f", [P, hid], dt.float32, kind="ExternalInput")
    iota = nc.dram_tensor("iota", [P, P], DT, kind="ExternalInput")
    out = nc.dram_tensor("out", [npc, hid], dt.float32, kind="ExternalOutput")

    n_lo = min(lo_split, n_nodes)

    with tile.TileContext(nc) as tc:
        with (
            tc.tile_pool(name="const", bufs=1) as cpool,
            tc.tile_pool(name="work", bufs=3) as pool,
            tc.tile_pool(name="psum", bufs=2, space="PSUM") as psum,
            tc.tile_pool(name="dram", bufs=1, space="DRAM") as dram,
        ):
            idx1_t = cpool.tile([P, tiles * cpt1 * P // 16], dt.int16)
            meta1_t = cpool.tile([P, tiles * cpt1 * 2], dt.float32)
            idx2_t = cpool.tile([P, tiles * cpt2 * P // 16], dt.int16)
            meta2_t = cpool.tile([P, tiles * cpt2 * 2], dt.float32)
            w1_t = cpool.tile([P, fb1 * hid], DT)
            w2_t = cpool.tile([P, fb2 * hid], DT)
            b1_t = cpool.tile([P, hid], dt.float32)
            b2_t = cpool.tile([P, hid], dt.float32)
            iota_t = cpool.tile([P, P], DT)
            ident_t = cpool.tile([P, P], DT)
            nc.sync.dma_start(idx1_t[:], idx1[:])
            nc.sync.dma_start(meta1_t[:], meta1[:])
            nc.sync.dma_start(idx2_t[:], idx2[:])
            nc.sync.dma_start(meta2_t[:], meta2[:])
            nc.sync.dma_start(w1_t[:], w1t[:])
            nc.sync.dma_start(w2_t[:], w2t[:])
            nc.sync.dma_start(b1_t[:], b1f[:])
            nc.sync.dma_start(b2_t[:], b2f[:])
            nc.sync.dma_start(iota_t[:], iota[:])
            make_identity(nc, ident_t[:])

            contrib_chunks = None
            hfull = None

            def layer(li, src_dram, n_src, feat, fbk, CL, CH, idx_t, meta_t,
                      wt_tile, bias_tile, dst_write, tile_done=None):
                """One GCN layer over all tiles of this core's dst range.

                Aggregation: agg[dst, :] += S_ch.T.T @ G_ch with S stationary
                (one weight load per chunk, wide moving operand), then PE
                transpose to get the dense matmul's lhsT."""
                cpt = CL + CH
                s_lo = min(lo_split, n_src)
                for t in range(tiles):
                    rows = last_rows if t == tiles - 1 else P
                    g_t = pool.tile([P, cpt, feat], DT, tag=f"g{li}")
                    icol = t * cpt * P // 16
                    if not no_gather:
                        nc.gpsimd.dma_gather(
                            g_t[:, :CL, :],
                            src_dram[0:s_lo, :],
                            idx_t[:, icol : icol + CL * P // 16],
                            CL * P, CL * P, feat,
                            queue_num=t % NQ,
                        )
                        if CH > 0:
                            nc.gpsimd.dma_gather(
                                g_t[:, CL:, :],
                                src_dram[s_lo:n_src, :],
                                idx_t[:, icol + CL * P // 16 : icol + cpt * P // 16],
                                CH * P, CH * P, feat,
                                queue_num=t % NQ,
                            )
                    s_t = pool.tile([P, cpt, P], DT, tag=f"s{li}")
                    for ch in range(cpt):
                        m = t * cpt + ch
                        nc.vector.tensor_scalar(
                            out=s_t[:, ch, :],
                            in0=iota_t[:],
                            scalar1=meta_t[:, 2 * m : 2 * m + 1],
                            scalar2=meta_t[:, 2 * m + 1 : 2 * m + 2],
                            op0=mybir.AluOpType.is_equal,
                            op1=mybir.AluOpType.mult,
                        )
                    agg = psum.tile([P, feat], dt.float32, tag=f"agg{li}")
                    for ch in range(cpt):
                        nc.tensor.matmul(
                            agg[:],
                            lhsT=s_t[:, ch, :],
                            rhs=g_t[:, ch, :],
                            start=(ch == 0),
                            stop=(ch == cpt - 1),
                        )
                    agg_sb = pool.tile([P, feat], DT, tag=f"asb{li}")
                    nc.scalar.copy(agg_sb[:], agg[:])
                    at_sb = pool.tile([P, fbk * P], DT, tag=f"at{li}")
                    for f in range(fbk):
                        tp = psum.tile([P, P], DT, tag="tp")
                        nc.tensor.transpose(
                            tp[:], in_=agg_sb[:, f * P : (f + 1) * P],
                            identity=ident_t[:],
                        )
                        nc.scalar.copy(at_sb[:, f * P : (f + 1) * P], tp[:])
                    h_ps = psum.tile([P, hid], dt.float32, tag="hps")
                    for f in range(fbk):
                        nc.tensor.matmul(
                            h_ps[:],
                            lhsT=at_sb[:, f * P : (f + 1) * P],
                            rhs=wt_tile[:, f * hid : (f + 1) * hid],
                            start=(f == 0),
                            stop=(f == fbk - 1),
                        )
                    nc.vector.tensor_add(h_ps[:], h_ps[:], bias_tile[:])
                    dst_write(t, rows, h_ps)
                    if tile_done is not None:
                        tile_done(t)

            def write_h(t, rows, h_ps):
                # PReLU(x) = max(x,0) + a*min(x,0), on DVE (sim-portable)
                h_sb = pool.tile([P, hid], DT, tag="h1")
                neg = pool.tile([P, hid], DT, tag="hneg")
                nc.vector.tensor_scalar(
                    out=h_sb[:], in0=h_ps[:], scalar1=0.0, scalar2=None,
                    op0=mybir.AluOpType.max,
                )
                nc.vector.tensor_scalar(
                    out=neg[:], in0=h_ps[:], scalar1=0.0, scalar2=float(prelu_a),
                    op0=mybir.AluOpType.min, op1=mybir.AluOpType.mult,
                )
                nc.vector.tensor_add(h_sb[:], h_sb[:], neg[:])
                c = t // TG
                roff = (t - c * TG) * P
                nc.sync.dma_start(
                    contrib_chunks[c][roff : roff + rows, :], h_sb[:rows, :]
                )

            def write_out(t, rows, h_ps):
                o_sb = pool.tile([P, hid], dt.float32, tag="o2")
                nc.vector.tensor_copy(o_sb[:], h_ps[:])
                nc.sync.dma_start(out[t * P : t * P + rows, :], o_sb[:rows, :])

            nchk = len(csizes)
            for _rep in range(reps):
                contrib_chunks = [
                    dram.tile([csizes[c], hid], DT, tag=f"contrib{c}_{_rep}",
                              name=f"contrib{c}_{_rep}")
                    for c in range(nchk)
                ]
                hfull = dram.tile([n_nodes, hid], DT, tag=f"hfull{_rep}",
                                  name=f"hfull{_rep}")

                def l1_tile_done(t):
                    # fire the chunk's AllGather as soon as its tiles are done
                    if profile_mode:
                        return
                    c = t // TG
                    if t == min((c + 1) * TG, tiles) - 1:
                        nc.gpsimd.collective_compute(
                            "AllGather",
                            mybir.AluOpType.bypass,
                            replica_groups=[list(range(n_cores))],
                            ins=[contrib_chunks[c].opt()],
                            outs=[
                                hfull[
                                    int(out_base[c]) : int(out_base[c])
                                    + n_cores * csizes[c],
                                    :,
                                ]
                            ],
                        )

                layer(1, x16, n_nodes, in_ch, fb1, CL1, CH1, idx1_t, meta1_t,
                      w1_t, b1_t, write_h, tile_done=l1_tile_done)
                layer(2, hfull, n_nodes, hid, fb2, CL2, CH2, idx2_t, meta2_t,
                      w2_t, b2_t, write_out)
    nc.compile()
    return nc


# --------------------------------------------------------------------- runner
def _run(inputs, n_nodes, in_ch, hid, n_cores, lo_split):
    _apply_patches()
    from concourse.bass_utils import run_bass_kernel_spmd

    x = np.asarray(inputs["x"], np.float32)
    W1 = np.asarray(inputs["W1"], np.float32)
    W2 = np.asarray(inputs["W2"], np.float32)
    b1 = np.asarray(inputs["b1"], np.float32)
    b2 = np.asarray(inputs["b2"], np.float32)
    prelu_a = float(np.asarray(inputs["prelu_a"]))

    per_ct, tiles, npc = _edge_buckets(
        inputs["edge_index"], inputs["edge_weight"], n_nodes, n_cores
    )
    gpos = _gpos_map(n_nodes, n_cores, tiles, npc)
    idx1a, meta1a, CL1, CH1 = _pack(per_ct, n_cores, tiles, None, lo_split)
    idx2a, meta2a, CL2, CH2 = _pack(per_ct, n_cores, tiles, gpos, lo_split)

    nc = _build(n_nodes, in_ch, hid, n_cores, lo_split, (CL1, CH1), (CL2, CH2),
                tiles, npc, prelu_a)

    npdt = np.float16 if FP16 else np.float32
    fb1 = in_ch // P
    fb2 = hid // P
    x16 = x.astype(npdt)
    w1t = W1.T.astype(npdt).reshape(fb1, P, hid).transpose(1, 0, 2).reshape(P, fb1 * hid)
    w2t = W2.T.astype(npdt).reshape(fb2, P, hid).transpose(1, 0, 2).reshape(P, fb2 * hid)
    b1f = np.tile(b1[None, :], (P, 1)).astype(np.float32)
    b2f = np.tile(b2[None, :], (P, 1)).astype(np.float32)
    iota = np.tile(np.arange(P, dtype=npdt)[None, :], (P, 1))

    in_maps = [
        {
            "x16": x16,
            "idx1": idx1a[c], "meta1": meta1a[c],
            "idx2": idx2a[c], "meta2": meta2a[c],
            "w1t": w1t, "w2t": w2t, "b1f": b1f, "b2f": b2f, "iota": iota,
        }
        for c in range(n_cores)
    ]
    res = run_bass_kernel_spmd(nc, in_maps, core_ids=list(range(n_cores)))
    outp = np.concatenate([res.results[c]["out"] for c in range(n_cores)], axis=0)
    return outp[:n_nodes]


def kernel(x, edge_index, edge_weight, W1, b1, W2, b2, prelu_a):
    inputs = dict(
        x=x, edge_index=edge_index, edge_weight=edge_weight,
        W1=W1, b1=b1, W2=W2, b2=b2, prelu_a=prelu_a,
    )
    return _run(inputs, N_NODES, IN_CH, HID, N_CORES, LO_SPLIT)


# ------------------------------------------------------------------- benchmark
def benchmark(inputs, n_iter=4, reps=(1, 3), profile_mode=False, no_gather=False):
    """Estimate pure device time of one kernel body via a replication delta:
    build the program with the body repeated r times; wall(r2) - wall(r1)
    cancels transfer/dispatch overhead. Returns ns per body."""
    import time
    _apply_patches()
    from concourse.bass_utils import run_bass_kernel_spmd

    x = np.asarray(inputs["x"], np.float32)
    prelu_a = float(np.asarray(inputs["prelu_a"]))
    per_ct, tiles, npc = _edge_buckets(
        inputs["edge_index"], inputs["edge_weight"], N_NODES, N_CORES
    )
    gpos = _gpos_map(N_NODES, N_CORES, tiles, npc)
    idx1a, meta1a, CL1, CH1 = _pack(per_ct, N_CORES, tiles, None, LO_SPLIT)
    idx2a, meta2a, CL2, CH2 = _pack(per_ct, N_CORES, tiles, gpos, LO_SPLIT)
    print(f"CL1={CL1} CH1={CH1} CL2={CL2} CH2={CH2}")
    npdt = np.float16 if FP16 else np.float32
    fb1, fb2 = IN_CH // P, HID // P
    W1 = np.asarray(inputs["W1"], np.float32)
    W2 = np.asarray(inputs["W2"], np.float32)
    x16 = x.astype(npdt)
    w1t = W1.T.astype(npdt).reshape(fb1, P, HID).transpose(1, 0, 2).reshape(P, fb1 * HID)
    w2t = W2.T.astype(npdt).reshape(fb2, P, HID).transpose(1, 0, 2).reshape(P, fb2 * HID)
    b1f = np.tile(np.asarray(inputs["b1"], np.float32)[None, :], (P, 1))
    b2f = np.tile(np.asarray(inputs["b2"], np.float32)[None, :], (P, 1))
    iota = np.tile(np.arange(P, dtype=npdt)[None, :], (P, 1))
    in_maps = [
        {"x16": x16,
         "idx1": idx1a[c], "meta1": meta1a[c],
         "idx2": idx2a[c], "meta2": meta2a[c],
         "w1t": w1t, "w2t": w2t, "b1f": b1f, "b2f": b2f, "iota": iota}
        for c in range(N_CORES)
    ]
    walls = {}
    for r in reps:
        nc = _build(N_NODES, IN_CH, HID, N_CORES, LO_SPLIT, (CL1, CH1), (CL2, CH2),
                    tiles, npc, prelu_a, reps=r, profile_mode=profile_mode,
                    no_gather=no_gather)
        ts = _timed_device_runs(nc, in_maps, n_iter)
        walls[r] = ts
        print(f"reps={r}: walls {['%.4f' % t for t in ts]}")
    r1, r2 = reps
    d = (min(walls[r2][1:]) - min(walls[r1][1:])) / (r2 - r1)
    return d * 1e9


def _timed_device_runs(nc, in_maps, n_iter):
    """Persistent-executable timed runs: inputs device-resident, outputs not
    fetched (block_until_ready only), so per-call wall ~= dispatch + exec."""
    import time
    import jax
    import jax.numpy as jnp
    from jax.sharding import Mesh, PartitionSpec, NamedSharding
    from jax.experimental.shard_map import shard_map
    import concourse.mybir as mybir
    from concourse.bass2jax import (
        install_neuronx_cc_hook, _bass_exec_p, partition_id_tensor,
    )

    install_neuronx_cc_hook()
    n_cores = len(in_maps)
    in_names, out_names, out_avals = [], [], []
    partition_name = nc.partition_id_tensor.name if nc.partition_id_tensor else None
    for alloc in nc.m.functions[0].allocations:
        if not isinstance(alloc, mybir.MemoryLocationSet):
            continue
        name = alloc.memorylocations[0].name
        if alloc.kind == "ExternalInput":
            if name != partition_name:
                in_names.append(name)
        elif alloc.kind == "ExternalOutput":
            out_names.append(name)
            out_avals.append(
                jax.core.ShapedArray(tuple(alloc.tensor_shape), mybir.dt.np(alloc.dtype))
            )
    n_params = len(in_names)
    all_in_names = in_names + out_names
    if partition_name is not None:
        all_in_names = all_in_names + [partition_name]

    def _body(*args):
        operands = list(args)
        if partition_name is not None:
            operands.append(partition_id_tensor())
        return tuple(
            _bass_exec_p.bind(
                *operands,
                out_avals=tuple(out_avals),
                in_names=tuple(all_in_names),
                out_names=tuple(out_names),
                lowering_input_output_aliases=(),
                sim_require_finite=True,
                sim_require_nnan=True,
                nc=nc,
            )
        )

    devices = jax.devices()[:n_cores]
    mesh = Mesh(np.asarray(devices), ("core",))
    spec = NamedSharding(mesh, PartitionSpec("core"))
    n_outs = len(out_names)
    donate = tuple(range(n_params, n_params + n_outs))
    sharded = jax.jit(
        shard_map(
            _body, mesh=mesh,
            in_specs=(PartitionSpec("core"),) * (n_params + n_outs),
            out_specs=(PartitionSpec("core"),) * n_outs,
            check_rep=False,
        ),
        donate_argnums=donate, keep_unused=True,
    )
    dev_in = [
        jax.device_put(
            np.concatenate([np.asarray(in_maps[c][nm]) for c in range(n_cores)], axis=0),
            spec,
        )
        for nm in in_names
    ]
    zero_shapes = [(n_cores * a.shape[0], *a.shape[1:]) for a in out_avals]

    def make_zeros():
        return [
            jax.device_put(jnp.zeros(s, a.dtype), spec)
            for s, a in zip(zero_shapes, out_avals)
        ]

    # Chained async timing: feed call k's outputs back as call k+1's donated
    # output buffers, block once at the end — dispatch overhead pipelines and
    # amortizes across the chain.
    n_chain = 20
    outs = tuple(make_zeros())
    outs = sharded(*dev_in, *outs)  # warmup + compile
    jax.block_until_ready(outs)
    ts = []
    for i in range(n_iter + 1):
        t0 = time.monotonic()
        for _ in range(n_chain):
            outs = sharded(*dev_in, *outs)
        jax.block_until_ready(outs)
        dt_s = (time.monotonic() - t0) / n_chain
        if i > 0:
            ts.append(dt_s)
    return ts


# revision 30
# speedup vs baseline: 1.6585x; 1.6585x over previous
"""Trainium2 Bass kernel for nn_CheckinEncoder (2-layer GCN, PReLU between).

Math (per GCNConv layer, PyG semantics):
    deg[d]  = sum_{e: dst_e=d} w_e + 1                (weighted in-degree + self loop)
    dis     = deg^{-1/2}
    norm_e  = dis[src_e] * w_e * dis[dst_e]           (self loop: 1/deg[d])
    agg     = scatter_add(norm_e * x[src_e] -> dst_e) (aggregate-first; linear
    h       = agg @ W.T + b                            and aggregation commute)

Sharding: dst nodes row-partitioned across 8 cores (6250 each). Each core
owns the edges into its nodes. Layer 1 gathers from a replicated fp16 copy
of x; between layers an on-device AllGather replicates h; layer 2 gathers
from that. Aggregation is done on the TensorEngine: for each 128-edge chunk
a selection matrix S[e, j] = (j == dst_local_e) * norm_e is built on the
VectorEngine (iota compare) and aggT[f, dst] += G[:, f].T @ S accumulates in
PSUM directly in the transposed layout the dense matmul needs as lhsT.

dma_gather indices are int16, so source windows are split at row 32768
("lo": src < 32768, base 0; "hi": src >= 32768, base 32768). Per (tile,
half) the chunk count is fixed globally (max over cores/tiles) and padded
with (idx=0, norm=0) so the single SPMD program fits every core.
"""

import numpy as np

# ---------------------------------------------------------------- problem dims
N_NODES = 50000
IN_CH = 256
HID = 512
N_CORES = 8
P = 128
LO_SPLIT = 32768
FP16 = True  # gather/matmul datapath dtype (fp32 accumulation throughout)
NQ = 4       # SWDGE queues used round-robin for gathers


# ------------------------------------------------------------------ tile patch
# This container's walrus accepts at most 1 sync wait per instruction
# (2 for EventSemaphore); Tile can emit more. Two fixes: split the kernel-tail
# drain's waits across nops, and legalize the final BIR by hoisting excess
# waits onto inserted NoOps (same engine, just before the instruction).
_PATCHED = False


def _apply_patches():
    global _PATCHED
    if _PATCHED:
        return
    _PATCHED = True
    import concourse.mybir as mybir
    import concourse.tile as tile
    import concourse.bass2jax as bass2jax
    import concourse.bass_utils as bass_utils
    from concourse.vector_clock import ScopedClock

    def _patched_drain_and_barrier(self, tick_clock, wait_clock):
        nc = self.nc
        drain_inst = nc.sync.drain()
        wait_clock.add_sem_waits(
            drain_inst.ins, ScopedClock({None: tick_clock.global_clock})
        )
        waits = list(drain_inst.ins.sync_info.on_wait)
        if len(waits) > 1:
            drain_inst.ins.sync_info.on_wait = waits[:1]
            for w in waits[1:]:
                nop = nc.sync.nop(nofuse=True, hint="drain_split_wait")
                if nop.ins.sync_info is None:
                    nop.ins.sync_info = mybir.SyncInfo(on_wait=[w], on_update=[])
                else:
                    nop.ins.sync_info.on_wait = [w]
        nc.all_engine_barrier()
        assert self.sems is not None
        popped = nc._tile_sem_poison_stack.pop()
        assert popped is self._sem_poison
        nc.clear_and_free_semaphores(list(self.sems.allocated().values()))
        nc.all_engine_barrier()

    tile.TileContext._drain_and_barrier = _patched_drain_and_barrier

    def _legalize_bir_json(bir_bytes):
        import orjson

        m = orjson.loads(bir_bytes)
        for fn in m.get("functions", []):
            for blk in fn.get("blocks", []):
                out = []
                for inst in blk.get("instructions", []):
                    si = inst.get("sync_info")
                    cap = 2 if inst.get("opcode") == "EventSemaphore" else 1
                    if si and len(si.get("on_wait") or []) > cap:
                        waits = si["on_wait"]
                        for k, w in enumerate(waits[:-cap]):
                            out.append(
                                {
                                    "debug": inst.get("debug", 0),
                                    "engine": inst["engine"],
                                    "ins": [],
                                    "outs": [],
                                    "name": f"{inst['name']}-lw{k}",
                                    "opcode": "NoOp",
                                    "sync_info": {"on_update": [], "on_wait": [w]},
                                }
                            )
                        si["on_wait"] = waits[-cap:]
                    out.append(inst)
                blk["instructions"] = out
        return orjson.dumps(m)

    orig = bass_utils.compile_bir_kernel

    def _wrapped(bir_json, tmpdir, neff_name="file.neff", **kw):
        return orig(_legalize_bir_json(bir_json), tmpdir, neff_name, **kw)

    bass_utils.compile_bir_kernel = _wrapped
    bass2jax.compile_bir_kernel = _wrapped


# ------------------------------------------------------------ host preprocessing
def _edge_buckets(edge_index, edge_weight, n_nodes, n_cores):
    """Bucket (src, dst_local, norm) per (core, tile), self loops included."""
    npc = n_nodes // n_cores
    tiles = (npc + P - 1) // P

    src = np.asarray(edge_index[0], dtype=np.int64)
    dst = np.asarray(edge_index[1], dtype=np.int64)
    w = np.asarray(edge_weight, dtype=np.float32)

    deg = np.bincount(dst, weights=w.astype(np.float64), minlength=n_nodes)
    deg = deg.astype(np.float32) + 1.0  # + self loop weight
    dis = 1.0 / np.sqrt(deg)
    norm = (dis[src] * w * dis[dst]).astype(np.float32)

    loop = np.arange(n_nodes, dtype=np.int64)
    src = np.concatenate([src, loop])
    dst = np.concatenate([dst, loop])
    norm = np.concatenate([norm, (dis * dis).astype(np.float32)])

    core_of = dst // npc
    tile_of = (dst % npc) // P
    dloc = (dst % npc) % P

    per_ct = {}
    for c in range(n_cores):
        cm = core_of == c
        for t in range(tiles):
            m = cm & (tile_of == t)
            per_ct[(c, t)] = (src[m], dloc[m], norm[m])
    return per_ct, tiles, npc


def _pack(per_ct, n_cores, tiles, srcmap, lo_split):
    """Pack bucketed edges into fixed-shape gather idx + (dst,norm) meta
    arrays. `srcmap` maps original src node id -> gather row id."""
    max_lo, max_hi = 1, 0
    split = {}
    for key, (s, d, n) in per_ct.items():
        g = srcmap[s] if srcmap is not None else s
        order = np.argsort(g, kind="stable")
        g, d2, n2 = g[order], d[order], n[order]
        lo = g < lo_split
        split[key] = (g[lo], d2[lo], n2[lo], g[~lo] - lo_split, d2[~lo], n2[~lo])
        max_lo = max(max_lo, int(lo.sum()))
        max_hi = max(max_hi, int((~lo).sum()))

    CL = (max_lo + P - 1) // P
    CH = (max_hi + P - 1) // P
    cpt = CL + CH

    sdt = np.float16 if FP16 else np.float32
    idx_arrays, s_arrays = [], []
    for c in range(n_cores):
        idx_flat = np.zeros((tiles * cpt * P,), np.int16)
        smat = np.zeros((P, tiles * cpt * P), sdt)
        for t in range(tiles):
            slo, dlo, nlo, shi, dhi, nhi = split[(c, t)]
            base = t * cpt * P
            idx_flat[base : base + len(slo)] = slo.astype(np.int16)
            idx_flat[base + CL * P : base + CL * P + len(shi)] = shi.astype(np.int16)
            for off, (dd, nn) in ((0, (dlo, nlo)), (CL * P, (dhi, nhi))):
                if len(dd) == 0:
                    continue
                i = np.arange(len(dd))
                part = i % P
                col = base + off + (i // P) * P + dd.astype(np.int64)
                smat[part, col] = nn.astype(sdt)
        idxw = idx_flat.reshape(-1, 16).T
        idx_arrays.append(np.tile(idxw, (8, 1)).copy())
        s_arrays.append(smat)
    return idx_arrays, s_arrays, CL, CH


# AllGather chunking: TG tiles of h per collective chunk, overlapped with
# remaining layer-1 compute. hfull layout is chunk-major: chunk c holds
# [rank 0 rows | rank 1 rows | ...]; gpos() maps node id -> hfull row.
TG = 7


def _chunk_sizes(tiles, npc):
    sizes = []
    for c0 in range(0, tiles, TG):
        lo = c0 * P
        hi = min((c0 + TG) * P, npc)
        sizes.append(hi - lo)
    return sizes


def _gpos_map(n_nodes, n_cores, tiles, npc):
    sizes = _chunk_sizes(tiles, npc)
    out_base = np.cumsum([0] + [8 * s for s in sizes])[:-1]
    n = np.arange(n_nodes, dtype=np.int64)
    r = n // npc
    l = n % npc
    c = np.minimum(l // (TG * P), len(sizes) - 1)
    s_c = np.asarray(sizes, np.int64)[c]
    return out_base[c] + r * s_c + (l - c * TG * P)


# ------------------------------------------------------------------ bass kernel
def _build(n_nodes, in_ch, hid, n_cores, lo_split, CLH1, CLH2, tiles, npc, prelu_a,
           reps=1, profile_mode=False, no_gather=False):
    import concourse.bacc as bacc
    import concourse.mybir as mybir
    import concourse.tile as tile

    from concourse.masks import make_identity

    dt = mybir.dt
    DT = dt.float16 if FP16 else dt.float32
    CL1, CH1 = CLH1
    CL2, CH2 = CLH2
    cpt1 = CL1 + CH1
    cpt2 = CL2 + CH2
    fb1 = in_ch // P   # feature blocks, layer-1 aggregation
    fb2 = hid // P
    last_rows = npc - (tiles - 1) * P
    csizes = _chunk_sizes(tiles, npc)
    out_base = np.cumsum([0] + [n_cores * s for s in csizes])[:-1]

    nc = bacc.Bacc(
        "TRN2", target_bir_lowering=False, num_devices=n_cores,
        num_swdge_queues=NQ,
    )
    x16 = nc.dram_tensor("x16", [n_nodes, in_ch], DT, kind="ExternalInput")
    idx1 = nc.dram_tensor("idx1", [P, tiles * cpt1 * P // 16], dt.int16, kind="ExternalInput")
    s1d = nc.dram_tensor("s1d", [P, tiles * cpt1 * P], DT, kind="ExternalInput")
    idx2 = nc.dram_tensor("idx2", [P, tiles * cpt2 * P // 16], dt.int16, kind="ExternalInput")
    s2d = nc.dram_tensor("s2d", [P, tiles * cpt2 * P], DT, kind="ExternalInput")
    w1t = nc.dram_tensor("w1t", [P, fb1 * hid], DT, kind="ExternalInput")
    w2t = nc.dram_tensor("w2t", [P, fb2 * hid], DT, kind="ExternalInput")
    b1f = nc.dram_tensor("b1f", [P, hid], dt.float32, kind="ExternalInput")
    b2f = nc.dram_tensor("b2f", [P, hid], dt.float32, kind="ExternalInput")
    iota = nc.dram_tensor("iota", [P, P], DT, kind="ExternalInput")
    out = nc.dram_tensor("out", [npc, hid], dt.float32, kind="ExternalOutput")

    n_lo = min(lo_split, n_nodes)

    with tile.TileContext(nc) as tc:
        with (
            tc.tile_pool(name="const", bufs=1) as cpool,
            tc.tile_pool(name="work", bufs=3) as pool,
            tc.tile_pool(name="psum", bufs=2, space="PSUM") as psum,
            tc.tile_pool(name="dram", bufs=1, space="DRAM") as dram,
        ):
            idx1_t = cpool.tile([P, tiles * cpt1 * P // 16], dt.int16)
            idx2_t = cpool.tile([P, tiles * cpt2 * P // 16], dt.int16)
            w1_t = cpool.tile([P, fb1 * hid], DT)
            w2_t = cpool.tile([P, fb2 * hid], DT)
            b1_t = cpool.tile([P, hid], dt.float32)
            b2_t = cpool.tile([P, hid], dt.float32)
            iota_t = cpool.tile([P, P], DT)
            ident_t = cpool.tile([P, P], DT)
            nc.sync.dma_start(idx1_t[:], idx1[:])
            nc.sync.dma_start(idx2_t[:], idx2[:])
            nc.sync.dma_start(w1_t[:], w1t[:])
            nc.sync.dma_start(w2_t[:], w2t[:])
            nc.sync.dma_start(b1_t[:], b1f[:])
            nc.sync.dma_start(b2_t[:], b2f[:])
            nc.sync.dma_start(iota_t[:], iota[:])
            make_identity(nc, ident_t[:])

            contrib_chunks = None
            hfull = None

            def layer(li, src_dram, n_src, feat, fbk, CL, CH, idx_t, s_dram,
                      wt_tile, bias_tile, dst_write, tile_done=None):
                """One GCN layer over all tiles of this core's dst range.

                Aggregation: agg[dst, :] += S_ch.T.T @ G_ch with S stationary
                (one weight load per chunk, wide moving operand), then PE
                transpose to get the dense matmul's lhsT."""
                cpt = CL + CH
                s_lo = min(lo_split, n_src)
                for t in range(tiles):
                    rows = last_rows if t == tiles - 1 else P
                    g_t = pool.tile([P, cpt, feat], DT, tag=f"g{li}")
                    icol = t * cpt * P // 16
                    if not no_gather:
                        nc.gpsimd.dma_gather(
                            g_t[:, :CL, :],
                            src_dram[0:s_lo, :],
                            idx_t[:, icol : icol + CL * P // 16],
                            CL * P, CL * P, feat,
                            queue_num=t % NQ,
                        )
                        if CH > 0:
                            nc.gpsimd.dma_gather(
                                g_t[:, CL:, :],
                                src_dram[s_lo:n_src, :],
                                idx_t[:, icol + CL * P // 16 : icol + cpt * P // 16],
                                CH * P, CH * P, feat,
                                queue_num=t % NQ,
                            )
                    s_t = pool.tile([P, cpt, P], DT, tag=f"s{li}")
                    nc.sync.dma_start(
                        s_t[:],
                        s_dram[:, t * cpt * P : (t + 1) * cpt * P],
                    )
                    agg = psum.tile([P, feat], dt.float32, tag=f"agg{li}")
                    for ch in range(cpt):
                        nc.tensor.matmul(
                            agg[:],
                            lhsT=s_t[:, ch, :],
                            rhs=g_t[:, ch, :],
                            start=(ch == 0),
                            stop=(ch == cpt - 1),
                        )
                    agg_sb = pool.tile([P, feat], DT, tag=f"asb{li}")
                    nc.scalar.copy(agg_sb[:], agg[:])
                    at_sb = pool.tile([P, fbk * P], DT, tag=f"at{li}")
                    for f in range(fbk):
                        tp = psum.tile([P, P], DT, tag="tp")
                        nc.tensor.transpose(
                            tp[:], in_=agg_sb[:, f * P : (f + 1) * P],
                            identity=ident_t[:],
                        )
                        nc.scalar.copy(at_sb[:, f * P : (f + 1) * P], tp[:])
                    h_ps = psum.tile([P, hid], dt.float32, tag="hps")
                    for f in range(fbk):
                        nc.tensor.matmul(
                            h_ps[:],
                            lhsT=at_sb[:, f * P : (f + 1) * P],
                            rhs=wt_tile[:, f * hid : (f + 1) * hid],
                            start=(f == 0),
                            stop=(f == fbk - 1),
                        )
                    nc.vector.tensor_add(h_ps[:], h_ps[:], bias_tile[:])
                    dst_write(t, rows, h_ps)
                    if tile_done is not None:
                        tile_done(t)

            def write_h(t, rows, h_ps):
                # PReLU(x) = max(x,0) + a*min(x,0), on DVE (sim-portable)
                h_sb = pool.tile([P, hid], DT, tag="h1")
                neg = pool.tile([P, hid], DT, tag="hneg")
                nc.vector.tensor_scalar(
                    out=h_sb[:], in0=h_ps[:], scalar1=0.0, scalar2=None,
                    op0=mybir.AluOpType.max,
                )
                nc.vector.tensor_scalar(
                    out=neg[:], in0=h_ps[:], scalar1=0.0, scalar2=float(prelu_a),
                    op0=mybir.AluOpType.min, op1=mybir.AluOpType.mult,
                )
                nc.vector.tensor_add(h_sb[:], h_sb[:], neg[:])
                c = t // TG
                roff = (t - c * TG) * P
                nc.sync.dma_start(
                    contrib_chunks[c][roff : roff + rows, :], h_sb[:rows, :]
                )

            def write_out(t, rows, h_ps):
                o_sb = pool.tile([P, hid], dt.float32, tag="o2")
                nc.vector.tensor_copy(o_sb[:], h_ps[:])
                nc.sync.dma_start(out[t * P : t * P + rows, :], o_sb[:rows, :])

            nchk = len(csizes)
            for _rep in range(reps):
                contrib_chunks = [
                    dram.tile([csizes[c], hid], DT, tag=f"contrib{c}_{_rep}",
                              name=f"contrib{c}_{_rep}")
                    for c in range(nchk)
                ]
                hfull = dram.tile([n_nodes, hid], DT, tag=f"hfull{_rep}",
                                  name=f"hfull{_rep}")

                def l1_tile_done(t):
                    # fire the chunk's AllGather as soon as its tiles are done
                    if profile_mode:
                        return
                    c = t // TG
                    if t == min((c + 1) * TG, tiles) - 1:
                        nc.gpsimd.collective_compute(
                            "AllGather",
                            mybir.AluOpType.bypass,
                            replica_groups=[list(range(n_cores))],
                            ins=[contrib_chunks[c].opt()],
                            outs=[
                                hfull[
                                    int(out_base[c]) : int(out_base[c])
                                    + n_cores * csizes[c],
                                    :,
                                ]
                            ],
                        )

                layer(1, x16, n_nodes, in_ch, fb1, CL1, CH1, idx1_t, s1d,
                      w1_t, b1_t, write_h, tile_done=l1_tile_done)
                layer(2, hfull, n_nodes, hid, fb2, CL2, CH2, idx2_t, s2d,
                      w2_t, b2_t, write_out)
    nc.compile()
    return nc


# --------------------------------------------------------------------- runner
def _run(inputs, n_nodes, in_ch, hid, n_cores, lo_split):
    _apply_patches()
    from concourse.bass_utils import run_bass_kernel_spmd

    x = np.asarray(inputs["x"], np.float32)
    W1 = np.asarray(inputs["W1"], np.float32)
    W2 = np.asarray(inputs["W2"], np.float32)
    b1 = np.asarray(inputs["b1"], np.float32)
    b2 = np.asarray(inputs["b2"], np.float32)
    prelu_a = float(np.asarray(inputs["prelu_a"]))

    per_ct, tiles, npc = _edge_buckets(
        inputs["edge_index"], inputs["edge_weight"], n_nodes, n_cores
    )
    gpos = _gpos_map(n_nodes, n_cores, tiles, npc)
    idx1a, s1a, CL1, CH1 = _pack(per_ct, n_cores, tiles, None, lo_split)
    idx2a, s2a, CL2, CH2 = _pack(per_ct, n_cores, tiles, gpos, lo_split)

    nc = _build(n_nodes, in_ch, hid, n_cores, lo_split, (CL1, CH1), (CL2, CH2),
                tiles, npc, prelu_a)

    npdt = np.float16 if FP16 else np.float32
    fb1 = in_ch // P
    fb2 = hid // P
    x16 = x.astype(npdt)
    w1t = W1.T.astype(npdt).reshape(fb1, P, hid).transpose(1, 0, 2).reshape(P, fb1 * hid)
    w2t = W2.T.astype(npdt).reshape(fb2, P, hid).transpose(1, 0, 2).reshape(P, fb2 * hid)
    b1f = np.tile(b1[None, :], (P, 1)).astype(np.float32)
    b2f = np.tile(b2[None, :], (P, 1)).astype(np.float32)
    iota = np.tile(np.arange(P, dtype=npdt)[None, :], (P, 1))

    in_maps = [
        {
            "x16": x16,
            "idx1": idx1a[c], "s1d": s1a[c],
            "idx2": idx2a[c], "s2d": s2a[c],
            "w1t": w1t, "w2t": w2t, "b1f": b1f, "b2f": b2f, "iota": iota,
        }
        for c in range(n_cores)
    ]
    res = run_bass_kernel_spmd(nc, in_maps, core_ids=list(range(n_cores)))
    outp = np.concatenate([res.results[c]["out"] for c in range(n_cores)], axis=0)
    return outp[:n_nodes]


def kernel(x, edge_index, edge_weight, W1, b1, W2, b2, prelu_a):
    inputs = dict(
        x=x, edge_index=edge_index, edge_weight=edge_weight,
        W1=W1, b1=b1, W2=W2, b2=b2, prelu_a=prelu_a,
    )
    return _run(inputs, N_NODES, IN_CH, HID, N_CORES, LO_SPLIT)


# ------------------------------------------------------------------- benchmark
def benchmark(inputs, n_iter=4, reps=(1, 3), profile_mode=False, no_gather=False):
    """Estimate pure device time of one kernel body via a replication delta:
    build the program with the body repeated r times; wall(r2) - wall(r1)
    cancels transfer/dispatch overhead. Returns ns per body."""
    import time
    _apply_patches()
    from concourse.bass_utils import run_bass_kernel_spmd

    x = np.asarray(inputs["x"], np.float32)
    prelu_a = float(np.asarray(inputs["prelu_a"]))
    per_ct, tiles, npc = _edge_buckets(
        inputs["edge_index"], inputs["edge_weight"], N_NODES, N_CORES
    )
    gpos = _gpos_map(N_NODES, N_CORES, tiles, npc)
    idx1a, s1a, CL1, CH1 = _pack(per_ct, N_CORES, tiles, None, LO_SPLIT)
    idx2a, s2a, CL2, CH2 = _pack(per_ct, N_CORES, tiles, gpos, LO_SPLIT)
    print(f"CL1={CL1} CH1={CH1} CL2={CL2} CH2={CH2}")
    npdt = np.float16 if FP16 else np.float32
    fb1, fb2 = IN_CH // P, HID // P
    W1 = np.asarray(inputs["W1"], np.float32)
    W2 = np.asarray(inputs["W2"], np.float32)
    x16 = x.astype(npdt)
    w1t = W1.T.astype(npdt).reshape(fb1, P, HID).transpose(1, 0, 2).reshape(P, fb1 * HID)
    w2t = W2.T.astype(npdt).reshape(fb2, P, HID).transpose(1, 0, 2).reshape(P, fb2 * HID)
    b1f = np.tile(np.asarray(inputs["b1"], np.float32)[None, :], (P, 1))
    b2f = np.tile(np.asarray(inputs["b2"], np.float32)[None, :], (P, 1))
    iota = np.tile(np.arange(P, dtype=npdt)[None, :], (P, 1))
    in_maps = [
        {"x16": x16,
         "idx1": idx1a[c], "s1d": s1a[c],
         "idx2": idx2a[c], "s2d": s2a[c],
         "w1t": w1t, "w2t": w2t, "b1f": b1f, "b2f": b2f, "iota": iota}
        for c in range(N_CORES)
    ]
    walls = {}
    for r in reps:
        nc = _build(N_NODES, IN_CH, HID, N_CORES, LO_SPLIT, (CL1, CH1), (CL2, CH2),
                    tiles, npc, prelu_a, reps=r, profile_mode=profile_mode,
                    no_gather=no_gather)
        ts = _timed_device_runs(nc, in_maps, n_iter)
        walls[r] = ts
        print(f"reps={r}: walls {['%.4f' % t for t in ts]}")
    r1, r2 = reps
    d = (min(walls[r2][1:]) - min(walls[r1][1:])) / (r2 - r1)
    return d * 1e9


def _timed_device_runs(nc, in_maps, n_iter):
    """Persistent-executable timed runs: inputs device-resident, outputs not
    fetched (block_until_ready only), so per-call wall ~= dispatch + exec."""
    import time
    import jax
    import jax.numpy as jnp
    from jax.sharding import Mesh, PartitionSpec, NamedSharding
    from jax.experimental.shard_map import shard_map
    import concourse.mybir as mybir
    from concourse.bass2jax import (
        install_neuronx_cc_hook, _bass_exec_p, partition_id_tensor,
    )

    install_neuronx_cc_hook()
    n_cores = len(in_maps)
    in_names, out_names, out_avals = [], [], []
    partition_name = nc.partition_id_tensor.name if nc.partition_id_tensor else None
    for alloc in nc.m.functions[0].allocations:
        if not isinstance(alloc, mybir.MemoryLocationSet):
            continue
        name = alloc.memorylocations[0].name
        if alloc.kind == "ExternalInput":
            if name != partition_name:
                in_names.append(name)
        elif alloc.kind == "ExternalOutput":
            out_names.append(name)
            out_avals.append(
                jax.core.ShapedArray(tuple(alloc.tensor_shape), mybir.dt.np(alloc.dtype))
            )
    n_params = len(in_names)
    all_in_names = in_names + out_names
    if partition_name is not None:
        all_in_names = all_in_names + [partition_name]

    def _body(*args):
        operands = list(args)
        if partition_name is not None:
            operands.append(partition_id_tensor())
        return tuple(
            _bass_exec_p.bind(
                *operands,
                out_avals=tuple(out_avals),
                in_names=tuple(all_in_names),
                out_names=tuple(out_names),
                lowering_input_output_aliases=(),
                sim_require_finite=True,
                sim_require_nnan=True,
                nc=nc,
            )
        )

    devices = jax.devices()[:n_cores]
    mesh = Mesh(np.asarray(devices), ("core",))
    spec = NamedSharding(mesh, PartitionSpec("core"))
    n_outs = len(out_names)
    donate = tuple(range(n_params, n_params + n_outs))
    sharded = jax.jit(
        shard_map(
            _body, mesh=mesh,
            in_specs=(PartitionSpec("core"),) * (n_params + n_outs),
            out_specs=(PartitionSpec("core"),) * n_outs,
            check_rep=False,
        ),
        donate_argnums=donate, keep_unused=True,
    )
    dev_in = [
        jax.device_put(
            np.concatenate([np.asarray(in_maps[c][nm]) for c in range(n_cores)], axis=0),
            spec,
        )
        for nm in in_names
    ]
    zero_shapes = [(n_cores * a.shape[0], *a.shape[1:]) for a in out_avals]

    def make_zeros():
        return [
            jax.device_put(jnp.zeros(s, a.dtype), spec)
            for s, a in zip(zero_shapes, out_avals)
        ]

    # Chained async timing: feed call k's outputs back as call k+1's donated
    # output buffers, block once at the end — dispatch overhead pipelines and
    # amortizes across the chain.
    n_chain = 20
    outs = tuple(make_zeros())
    outs = sharded(*dev_in, *outs)  # warmup + compile
    jax.block_until_ready(outs)
    ts = []
    for i in range(n_iter + 1):
        t0 = time.monotonic()
        for _ in range(n_chain):
            outs = sharded(*dev_in, *outs)
        jax.block_until_ready(outs)
        dt_s = (time.monotonic() - t0) / n_chain
        if i > 0:
            ts.append(dt_s)
    return ts


# revision 34
# speedup vs baseline: 2.0236x; 1.2201x over previous
"""Trainium2 Bass kernel for nn_CheckinEncoder (2-layer GCN, PReLU between).

Math (per GCNConv layer, PyG semantics):
    deg[d]  = sum_{e: dst_e=d} w_e + 1                (weighted in-degree + self loop)
    dis     = deg^{-1/2}
    norm_e  = dis[src_e] * w_e * dis[dst_e]           (self loop: 1/deg[d])
    agg     = scatter_add(norm_e * x[src_e] -> dst_e) (aggregate-first; linear
    h       = agg @ W.T + b                            and aggregation commute)

Sharding: dst nodes row-partitioned across 8 cores (6250 each). Each core
owns the edges into its nodes. Layer 1 gathers from a replicated fp16 copy
of x; between layers an on-device AllGather replicates h; layer 2 gathers
from that. Aggregation is done on the TensorEngine: for each 128-edge chunk
a selection matrix S[e, j] = (j == dst_local_e) * norm_e is built on the
VectorEngine (iota compare) and aggT[f, dst] += G[:, f].T @ S accumulates in
PSUM directly in the transposed layout the dense matmul needs as lhsT.

dma_gather indices are int16, so source windows are split at row 32768
("lo": src < 32768, base 0; "hi": src >= 32768, base 32768). Per (tile,
half) the chunk count is fixed globally (max over cores/tiles) and padded
with (idx=0, norm=0) so the single SPMD program fits every core.
"""

import numpy as np

# ---------------------------------------------------------------- problem dims
N_NODES = 50000
IN_CH = 256
HID = 512
N_CORES = 8
P = 128
LO_SPLIT = 32768
FP16 = True  # gather/matmul datapath dtype (fp32 accumulation throughout)
NQ = 4       # SWDGE queues used round-robin for gathers
WORK_BUFS = 3  # work tile pool depth (pipeline overlap)


# ------------------------------------------------------------------ tile patch
# This container's walrus accepts at most 1 sync wait per instruction
# (2 for EventSemaphore); Tile can emit more. Two fixes: split the kernel-tail
# drain's waits across nops, and legalize the final BIR by hoisting excess
# waits onto inserted NoOps (same engine, just before the instruction).
_PATCHED = False


def _apply_patches():
    global _PATCHED
    if _PATCHED:
        return
    _PATCHED = True
    import concourse.mybir as mybir
    import concourse.tile as tile
    import concourse.bass2jax as bass2jax
    import concourse.bass_utils as bass_utils
    from concourse.vector_clock import ScopedClock

    def _patched_drain_and_barrier(self, tick_clock, wait_clock):
        nc = self.nc
        drain_inst = nc.sync.drain()
        wait_clock.add_sem_waits(
            drain_inst.ins, ScopedClock({None: tick_clock.global_clock})
        )
        waits = list(drain_inst.ins.sync_info.on_wait)
        if len(waits) > 1:
            drain_inst.ins.sync_info.on_wait = waits[:1]
            for w in waits[1:]:
                nop = nc.sync.nop(nofuse=True, hint="drain_split_wait")
                if nop.ins.sync_info is None:
                    nop.ins.sync_info = mybir.SyncInfo(on_wait=[w], on_update=[])
                else:
                    nop.ins.sync_info.on_wait = [w]
        nc.all_engine_barrier()
        assert self.sems is not None
        popped = nc._tile_sem_poison_stack.pop()
        assert popped is self._sem_poison
        nc.clear_and_free_semaphores(list(self.sems.allocated().values()))
        nc.all_engine_barrier()

    tile.TileContext._drain_and_barrier = _patched_drain_and_barrier

    def _legalize_bir_json(bir_bytes):
        import orjson

        m = orjson.loads(bir_bytes)
        for fn in m.get("functions", []):
            for blk in fn.get("blocks", []):
                out = []
                for inst in blk.get("instructions", []):
                    si = inst.get("sync_info")
                    cap = 2 if inst.get("opcode") == "EventSemaphore" else 1
                    if si and len(si.get("on_wait") or []) > cap:
                        waits = si["on_wait"]
                        for k, w in enumerate(waits[:-cap]):
                            out.append(
                                {
                                    "debug": inst.get("debug", 0),
                                    "engine": inst["engine"],
                                    "ins": [],
                                    "outs": [],
                                    "name": f"{inst['name']}-lw{k}",
                                    "opcode": "NoOp",
                                    "sync_info": {"on_update": [], "on_wait": [w]},
                                }
                            )
                        si["on_wait"] = waits[-cap:]
                    out.append(inst)
                blk["instructions"] = out
        return orjson.dumps(m)

    orig = bass_utils.compile_bir_kernel

    def _wrapped(bir_json, tmpdir, neff_name="file.neff", **kw):
        return orig(_legalize_bir_json(bir_json), tmpdir, neff_name, **kw)

    bass_utils.compile_bir_kernel = _wrapped
    bass2jax.compile_bir_kernel = _wrapped


# ------------------------------------------------------------ host preprocessing
def _edge_buckets(edge_index, edge_weight, n_nodes, n_cores):
    """Bucket (src, dst_local, norm) per (core, tile), self loops included."""
    npc = n_nodes // n_cores
    tiles = (npc + P - 1) // P

    src = np.asarray(edge_index[0], dtype=np.int64)
    dst = np.asarray(edge_index[1], dtype=np.int64)
    w = np.asarray(edge_weight, dtype=np.float32)

    deg = np.bincount(dst, weights=w.astype(np.float64), minlength=n_nodes)
    deg = deg.astype(np.float32) + 1.0  # + self loop weight
    dis = 1.0 / np.sqrt(deg)
    norm = (dis[src] * w * dis[dst]).astype(np.float32)

    loop = np.arange(n_nodes, dtype=np.int64)
    src = np.concatenate([src, loop])
    dst = np.concatenate([dst, loop])
    norm = np.concatenate([norm, (dis * dis).astype(np.float32)])

    core_of = dst // npc
    tile_of = (dst % npc) // P
    dloc = (dst % npc) % P

    per_ct = {}
    for c in range(n_cores):
        cm = core_of == c
        for t in range(tiles):
            m = cm & (tile_of == t)
            per_ct[(c, t)] = (src[m], dloc[m], norm[m])
    return per_ct, tiles, npc


def _pack(per_ct, n_cores, tiles, srcmap, lo_split):
    """Pack bucketed edges into fixed-shape gather idx + (dst,norm) meta
    arrays. `srcmap` maps original src node id -> gather row id."""
    max_lo, max_hi = 1, 0
    split = {}
    for key, (s, d, n) in per_ct.items():
        g = srcmap[s] if srcmap is not None else s
        order = np.argsort(g, kind="stable")
        g, d2, n2 = g[order], d[order], n[order]
        lo = g < lo_split
        split[key] = (g[lo], d2[lo], n2[lo], g[~lo] - lo_split, d2[~lo], n2[~lo])
        max_lo = max(max_lo, int(lo.sum()))
        max_hi = max(max_hi, int((~lo).sum()))

    CL = (max_lo + P - 1) // P
    CH = (max_hi + P - 1) // P
    cpt = CL + CH

    idx_arrays, meta_arrays = [], []
    for c in range(n_cores):
        idx_flat = np.zeros((tiles * cpt * P,), np.int16)
        meta = np.zeros((P, tiles * cpt * 2), np.float32)
        for t in range(tiles):
            slo, dlo, nlo, shi, dhi, nhi = split[(c, t)]
            base = t * cpt * P
            idx_flat[base : base + len(slo)] = slo.astype(np.int16)
            idx_flat[base + CL * P : base + CL * P + len(shi)] = shi.astype(np.int16)
            dcol = np.zeros((cpt * P,), np.float32)
            ncol = np.zeros((cpt * P,), np.float32)
            dcol[: len(dlo)] = dlo
            ncol[: len(nlo)] = nlo
            dcol[CL * P : CL * P + len(dhi)] = dhi
            ncol[CL * P : CL * P + len(nhi)] = nhi
            for ch in range(cpt):
                m = t * cpt + ch
                meta[:, 2 * m] = dcol[ch * P : (ch + 1) * P]
                meta[:, 2 * m + 1] = ncol[ch * P : (ch + 1) * P]
        idxw = idx_flat.reshape(-1, 16).T
        idx_arrays.append(np.tile(idxw, (8, 1)).copy())
        meta_arrays.append(meta)
    return idx_arrays, meta_arrays, CL, CH


# AllGather chunking: TG tiles of h per collective chunk, overlapped with
# remaining layer-1 compute. hfull layout is chunk-major: chunk c holds
# [rank 0 rows | rank 1 rows | ...]; gpos() maps node id -> hfull row.
TG = 7


def _chunk_sizes(tiles, npc):
    sizes = []
    for c0 in range(0, tiles, TG):
        lo = c0 * P
        hi = min((c0 + TG) * P, npc)
        sizes.append(hi - lo)
    return sizes


def _gpos_map(n_nodes, n_cores, tiles, npc):
    sizes = _chunk_sizes(tiles, npc)
    out_base = np.cumsum([0] + [8 * s for s in sizes])[:-1]
    n = np.arange(n_nodes, dtype=np.int64)
    r = n // npc
    l = n % npc
    c = np.minimum(l // (TG * P), len(sizes) - 1)
    s_c = np.asarray(sizes, np.int64)[c]
    return out_base[c] + r * s_c + (l - c * TG * P)


# ------------------------------------------------------------------ bass kernel
def _build(n_nodes, in_ch, hid, n_cores, lo_split, CLH1, CLH2, tiles, npc, prelu_a,
           reps=1, profile_mode=False, no_gather=False):
    import concourse.bacc as bacc
    import concourse.mybir as mybir
    import concourse.tile as tile

    from concourse.masks import make_identity

    dt = mybir.dt
    DT = dt.float16 if FP16 else dt.float32
    CL1, CH1 = CLH1
    CL2, CH2 = CLH2
    cpt1 = CL1 + CH1
    cpt2 = CL2 + CH2
    fb1 = in_ch // P   # feature blocks, layer-1 aggregation
    fb2 = hid // P
    last_rows = npc - (tiles - 1) * P
    csizes = _chunk_sizes(tiles, npc)
    out_base = np.cumsum([0] + [n_cores * s for s in csizes])[:-1]

    nc = bacc.Bacc(
        "TRN2", target_bir_lowering=False, num_devices=n_cores,
        num_swdge_queues=NQ,
    )
    x16 = nc.dram_tensor("x16", [n_nodes, in_ch], DT, kind="ExternalInput")
    idx1 = nc.dram_tensor("idx1", [P, tiles * cpt1 * P // 16], dt.int16, kind="ExternalInput")
    meta1 = nc.dram_tensor("meta1", [P, tiles * cpt1 * 2], dt.float32, kind="ExternalInput")
    idx2 = nc.dram_tensor("idx2", [P, tiles * cpt2 * P // 16], dt.int16, kind="ExternalInput")
    meta2 = nc.dram_tensor("meta2", [P, tiles * cpt2 * 2], dt.float32, kind="ExternalInput")
    w1t = nc.dram_tensor("w1t", [P, fb1 * hid], DT, kind="ExternalInput")
    w2t = nc.dram_tensor("w2t", [P, fb2 * hid], DT, kind="ExternalInput")
    b1f = nc.dram_tensor("b1f", [P, hid], dt.float32, kind="ExternalInput")
    b2f = nc.dram_tensor("b2f", [P, hid], dt.float32, kind="ExternalInput")
    iota = nc.dram_tensor("iota", [P, P], DT, kind="ExternalInput")
    out = nc.dram_tensor("out", [npc, hid], dt.float32, kind="ExternalOutput")

    n_lo = min(lo_split, n_nodes)

    with tile.TileContext(nc) as tc:
        with (
            tc.tile_pool(name="const", bufs=1) as cpool,
            tc.tile_pool(name="work", bufs=WORK_BUFS) as pool,
            tc.tile_pool(name="psum", bufs=2, space="PSUM") as psum,
            tc.tile_pool(name="dram", bufs=1, space="DRAM") as dram,
        ):
            idx1_t = cpool.tile([P, tiles * cpt1 * P // 16], dt.int16)
            meta1_t = cpool.tile([P, tiles * cpt1 * 2], dt.float32)
            idx2_t = cpool.tile([P, tiles * cpt2 * P // 16], dt.int16)
            meta2_t = cpool.tile([P, tiles * cpt2 * 2], dt.float32)
            w1_t = cpool.tile([P, fb1 * hid], DT)
            w2_t = cpool.tile([P, fb2 * hid], DT)
            b1_t = cpool.tile([P, hid], dt.float32)
            b2_t = cpool.tile([P, hid], dt.float32)
            iota_t = cpool.tile([P, P], DT)
            ident_t = cpool.tile([P, P], DT)
            nc.sync.dma_start(idx1_t[:], idx1[:])
            nc.sync.dma_start(meta1_t[:], meta1[:])
            nc.sync.dma_start(idx2_t[:], idx2[:])
            nc.sync.dma_start(meta2_t[:], meta2[:])
            nc.sync.dma_start(w1_t[:], w1t[:])
            nc.sync.dma_start(w2_t[:], w2t[:])
            nc.sync.dma_start(b1_t[:], b1f[:])
            nc.sync.dma_start(b2_t[:], b2f[:])
            nc.sync.dma_start(iota_t[:], iota[:])
            make_identity(nc, ident_t[:])

            contrib_chunks = None
            hfull = None

            def layer(li, src_dram, n_src, feat, fbk, CL, CH, idx_t, meta_t,
                      wt_tile, bias_tile, dst_write, tile_done=None):
                """One GCN layer over all tiles of this core's dst range.

                Aggregation: agg[dst, :] += S_ch.T.T @ G_ch with S stationary
                (one weight load per chunk, wide moving operand), then PE
                transpose to get the dense matmul's lhsT."""
                cpt = CL + CH
                s_lo = min(lo_split, n_src)
                for t in range(tiles):
                    rows = last_rows if t == tiles - 1 else P
                    g_t = pool.tile([P, cpt, feat], DT, tag=f"g{li}")
                    icol = t * cpt * P // 16
                    if not no_gather:
                        nc.gpsimd.dma_gather(
                            g_t[:, :CL, :],
                            src_dram[0:s_lo, :],
                            idx_t[:, icol : icol + CL * P // 16],
                            CL * P, CL * P, feat,
                            queue_num=t % NQ,
                        )
                        if CH > 0:
                            nc.gpsimd.dma_gather(
                                g_t[:, CL:, :],
                                src_dram[s_lo:n_src, :],
                                idx_t[:, icol + CL * P // 16 : icol + cpt * P // 16],
                                CH * P, CH * P, feat,
                                queue_num=t % NQ,
                            )
                    s_t = pool.tile([P, cpt, P], DT, tag=f"s{li}")
                    for ch in range(cpt):
                        m = t * cpt + ch
                        nc.vector.tensor_scalar(
                            out=s_t[:, ch, :],
                            in0=iota_t[:],
                            scalar1=meta_t[:, 2 * m : 2 * m + 1],
                            scalar2=meta_t[:, 2 * m + 1 : 2 * m + 2],
                            op0=mybir.AluOpType.is_equal,
                            op1=mybir.AluOpType.mult,
                        )
                    agg = psum.tile([P, feat], dt.float32, tag=f"agg{li}")
                    for ch in range(cpt):
                        nc.tensor.matmul(
                            agg[:],
                            lhsT=s_t[:, ch, :],
                            rhs=g_t[:, ch, :],
                            start=(ch == 0),
                            stop=(ch == cpt - 1),
                        )
                    agg_sb = pool.tile([P, feat], DT, tag=f"asb{li}")
                    nc.scalar.copy(agg_sb[:], agg[:])
                    at_sb = pool.tile([P, fbk * P], DT, tag=f"at{li}")
                    for f in range(fbk):
                        tp = psum.tile([P, P], DT, tag="tp")
                        nc.tensor.transpose(
                            tp[:], in_=agg_sb[:, f * P : (f + 1) * P],
                            identity=ident_t[:],
                        )
                        nc.scalar.copy(at_sb[:, f * P : (f + 1) * P], tp[:])
                    h_ps = psum.tile([P, hid], dt.float32, tag="hps")
                    for f in range(fbk):
                        nc.tensor.matmul(
                            h_ps[:],
                            lhsT=at_sb[:, f * P : (f + 1) * P],
                            rhs=wt_tile[:, f * hid : (f + 1) * hid],
                            start=(f == 0),
                            stop=(f == fbk - 1),
                        )
                    nc.vector.tensor_add(h_ps[:], h_ps[:], bias_tile[:])
                    dst_write(t, rows, h_ps)
                    if tile_done is not None:
                        tile_done(t)

            def write_h(t, rows, h_ps):
                # PReLU(x) = max(x,0) + a*min(x,0), on DVE (sim-portable)
                h_sb = pool.tile([P, hid], DT, tag="h1")
                neg = pool.tile([P, hid], DT, tag="hneg")
                nc.vector.tensor_scalar(
                    out=h_sb[:], in0=h_ps[:], scalar1=0.0, scalar2=None,
                    op0=mybir.AluOpType.max,
                )
                nc.vector.tensor_scalar(
                    out=neg[:], in0=h_ps[:], scalar1=0.0, scalar2=float(prelu_a),
                    op0=mybir.AluOpType.min, op1=mybir.AluOpType.mult,
                )
                nc.vector.tensor_add(h_sb[:], h_sb[:], neg[:])
                c = t // TG
                roff = (t - c * TG) * P
                nc.sync.dma_start(
                    contrib_chunks[c][roff : roff + rows, :], h_sb[:rows, :]
                )

            def write_out(t, rows, h_ps):
                o_sb = pool.tile([P, hid], dt.float32, tag="o2")
                nc.vector.tensor_copy(o_sb[:], h_ps[:])
                nc.sync.dma_start(out[t * P : t * P + rows, :], o_sb[:rows, :])

            nchk = len(csizes)
            for _rep in range(reps):
                contrib_chunks = [
                    dram.tile([csizes[c], hid], DT, tag=f"contrib{c}_{_rep}",
                              name=f"contrib{c}_{_rep}")
                    for c in range(nchk)
                ]
                hfull = dram.tile([n_nodes, hid], DT, tag=f"hfull{_rep}",
                                  name=f"hfull{_rep}")

                def l1_tile_done(t):
                    # fire the chunk's AllGather as soon as its tiles are done
                    if profile_mode:
                        return
                    c = t // TG
                    if t == min((c + 1) * TG, tiles) - 1:
                        nc.gpsimd.collective_compute(
                            "AllGather",
                            mybir.AluOpType.bypass,
                            replica_groups=[list(range(n_cores))],
                            ins=[contrib_chunks[c].opt()],
                            outs=[
                                hfull[
                                    int(out_base[c]) : int(out_base[c])
                                    + n_cores * csizes[c],
                                    :,
                                ]
                            ],
                        )

                layer(1, x16, n_nodes, in_ch, fb1, CL1, CH1, idx1_t, meta1_t,
                      w1_t, b1_t, write_h, tile_done=l1_tile_done)
                layer(2, hfull, n_nodes, hid, fb2, CL2, CH2, idx2_t, meta2_t,
                      w2_t, b2_t, write_out)
    nc.compile()
    return nc


# --------------------------------------------------------------------- runner
def _run(inputs, n_nodes, in_ch, hid, n_cores, lo_split):
    _apply_patches()
    from concourse.bass_utils import run_bass_kernel_spmd

    x = np.asarray(inputs["x"], np.float32)
    W1 = np.asarray(inputs["W1"], np.float32)
    W2 = np.asarray(inputs["W2"], np.float32)
    b1 = np.asarray(inputs["b1"], np.float32)
    b2 = np.asarray(inputs["b2"], np.float32)
    prelu_a = float(np.asarray(inputs["prelu_a"]))

    per_ct, tiles, npc = _edge_buckets(
        inputs["edge_index"], inputs["edge_weight"], n_nodes, n_cores
    )
    gpos = _gpos_map(n_nodes, n_cores, tiles, npc)
    idx1a, meta1a, CL1, CH1 = _pack(per_ct, n_cores, tiles, None, lo_split)
    idx2a, meta2a, CL2, CH2 = _pack(per_ct, n_cores, tiles, gpos, lo_split)

    nc = _build(n_nodes, in_ch, hid, n_cores, lo_split, (CL1, CH1), (CL2, CH2),
                tiles, npc, prelu_a)

    npdt = np.float16 if FP16 else np.float32
    fb1 = in_ch // P
    fb2 = hid // P
    x16 = x.astype(npdt)
    w1t = W1.T.astype(npdt).reshape(fb1, P, hid).transpose(1, 0, 2).reshape(P, fb1 * hid)
    w2t = W2.T.astype(npdt).reshape(fb2, P, hid).transpose(1, 0, 2).reshape(P, fb2 * hid)
    b1f = np.tile(b1[None, :], (P, 1)).astype(np.float32)
    b2f = np.tile(b2[None, :], (P, 1)).astype(np.float32)
    iota = np.tile(np.arange(P, dtype=npdt)[None, :], (P, 1))

    in_maps = [
        {
            "x16": x16,
            "idx1": idx1a[c], "meta1": meta1a[c],
            "idx2": idx2a[c], "meta2": meta2a[c],
            "w1t": w1t, "w2t": w2t, "b1f": b1f, "b2f": b2f, "iota": iota,
        }
        for c in range(n_cores)
    ]
    res = run_bass_kernel_spmd(nc, in_maps, core_ids=list(range(n_cores)))
    outp = np.concatenate([res.results[c]["out"] for c in range(n_cores)], axis=0)
    return outp[:n_nodes]


def kernel(x, edge_index, edge_weight, W1, b1, W2, b2, prelu_a):
    inputs = dict(
        x=x, edge_index=edge_index, edge_weight=edge_weight,
        W1=W1, b1=b1, W2=W2, b2=b2, prelu_a=prelu_a,
    )
    return _run(inputs, N_NODES, IN_CH, HID, N_CORES, LO_SPLIT)


# ------------------------------------------------------------------- benchmark
def benchmark(inputs, n_iter=4, reps=(1, 3), profile_mode=False, no_gather=False):
    """Estimate pure device time of one kernel body via a replication delta:
    build the program with the body repeated r times; wall(r2) - wall(r1)
    cancels transfer/dispatch overhead. Returns ns per body."""
    import time
    _apply_patches()
    from concourse.bass_utils import run_bass_kernel_spmd

    x = np.asarray(inputs["x"], np.float32)
    prelu_a = float(np.asarray(inputs["prelu_a"]))
    per_ct, tiles, npc = _edge_buckets(
        inputs["edge_index"], inputs["edge_weight"], N_NODES, N_CORES
    )
    gpos = _gpos_map(N_NODES, N_CORES, tiles, npc)
    idx1a, meta1a, CL1, CH1 = _pack(per_ct, N_CORES, tiles, None, LO_SPLIT)
    idx2a, meta2a, CL2, CH2 = _pack(per_ct, N_CORES, tiles, gpos, LO_SPLIT)
    print(f"CL1={CL1} CH1={CH1} CL2={CL2} CH2={CH2}")
    npdt = np.float16 if FP16 else np.float32
    fb1, fb2 = IN_CH // P, HID // P
    W1 = np.asarray(inputs["W1"], np.float32)
    W2 = np.asarray(inputs["W2"], np.float32)
    x16 = x.astype(npdt)
    w1t = W1.T.astype(npdt).reshape(fb1, P, HID).transpose(1, 0, 2).reshape(P, fb1 * HID)
    w2t = W2.T.astype(npdt).reshape(fb2, P, HID).transpose(1, 0, 2).reshape(P, fb2 * HID)
    b1f = np.tile(np.asarray(inputs["b1"], np.float32)[None, :], (P, 1))
    b2f = np.tile(np.asarray(inputs["b2"], np.float32)[None, :], (P, 1))
    iota = np.tile(np.arange(P, dtype=npdt)[None, :], (P, 1))
    in_maps = [
        {"x16": x16,
         "idx1": idx1a[c], "meta1": meta1a[c],
         "idx2": idx2a[c], "meta2": meta2a[c],
         "w1t": w1t, "w2t": w2t, "b1f": b1f, "b2f": b2f, "iota": iota}
        for c in range(N_CORES)
    ]
    walls = {}
    for r in reps:
        nc = _build(N_NODES, IN_CH, HID, N_CORES, LO_SPLIT, (CL1, CH1), (CL2, CH2),
                    tiles, npc, prelu_a, reps=r, profile_mode=profile_mode,
                    no_gather=no_gather)
        ts = _timed_device_runs(nc, in_maps, n_iter)
        walls[r] = ts
        print(f"reps={r}: walls {['%.4f' % t for t in ts]}")
    r1, r2 = reps
    d = (min(walls[r2][1:]) - min(walls[r1][1:])) / (r2 - r1)
    return d * 1e9


def _timed_device_runs(nc, in_maps, n_iter):
    """Persistent-executable timed runs: inputs device-resident, outputs not
    fetched (block_until_ready only), so per-call wall ~= dispatch + exec."""
    import time
    import jax
    import jax.numpy as jnp
    from jax.sharding import Mesh, PartitionSpec, NamedSharding
    from jax.experimental.shard_map import shard_map
    import concourse.mybir as mybir
    from concourse.bass2jax import (
        install_neuronx_cc_hook, _bass_exec_p, partition_id_tensor,
    )

    install_neuronx_cc_hook()
    n_cores = len(in_maps)
    in_names, out_names, out_avals = [], [], []
    partition_name = nc.partition_id_tensor.name if nc.partition_id_tensor else None
    for alloc in nc.m.functions[0].allocations:
        if not isinstance(alloc, mybir.MemoryLocationSet):
            continue
        name = alloc.memorylocations[0].name
        if alloc.kind == "ExternalInput":
            if name != partition_name:
                in_names.append(name)
        elif alloc.kind == "ExternalOutput":
            out_names.append(name)
            out_avals.append(
                jax.core.ShapedArray(tuple(alloc.tensor_shape), mybir.dt.np(alloc.dtype))
            )
    n_params = len(in_names)
    all_in_names = in_names + out_names
    if partition_name is not None:
        all_in_names = all_in_names + [partition_name]

    def _body(*args):
        operands = list(args)
        if partition_name is not None:
            operands.append(partition_id_tensor())
        return tuple(
            _bass_exec_p.bind(
                *operands,
                out_avals=tuple(out_avals),
                in_names=tuple(all_in_names),
                out_names=tuple(out_names),
                lowering_input_output_aliases=(),
                sim_require_finite=True,
                sim_require_nnan=True,
                nc=nc,
            )
        )

    devices = jax.devices()[:n_cores]
    mesh = Mesh(np.asarray(devices), ("core",))
    spec = NamedSharding(mesh, PartitionSpec("core"))
    n_outs = len(out_names)
    donate = tuple(range(n_params, n_params + n_outs))
    sharded = jax.jit(
        shard_map(
            _body, mesh=mesh,
            in_specs=(PartitionSpec("core"),) * (n_params + n_outs),
            out_specs=(PartitionSpec("core"),) * n_outs,
            check_rep=False,
        ),
        donate_argnums=donate, keep_unused=True,
    )
    dev_in = [
        jax.device_put(
            np.concatenate([np.asarray(in_maps[c][nm]) for c in range(n_cores)], axis=0),
            spec,
        )
        for nm in in_names
    ]
    zero_shapes = [(n_cores * a.shape[0], *a.shape[1:]) for a in out_avals]

    def make_zeros():
        return [
            jax.device_put(jnp.zeros(s, a.dtype), spec)
            for s, a in zip(zero_shapes, out_avals)
        ]

    # Chained async timing: feed call k's outputs back as call k+1's donated
    # output buffers, block once at the end — dispatch overhead pipelines and
    # amortizes across the chain.
    n_chain = 20
    outs = tuple(make_zeros())
    outs = sharded(*dev_in, *outs)  # warmup + compile
    jax.block_until_ready(outs)
    ts = []
    for i in range(n_iter + 1):
        t0 = time.monotonic()
        for _ in range(n_chain):
            outs = sharded(*dev_in, *outs)
        jax.block_until_ready(outs)
        dt_s = (time.monotonic() - t0) / n_chain
        if i > 0:
            ts.append(dt_s)
    return ts
